# revision 40
# baseline (speedup 1.0000x reference)
"""MoE-routed multi-head attention kernel for 8 Trainium2 NeuronCores. v3

Problem shape (hardcoded):
  query/key/value: [4, 2048, 512] f32
  Wg [512,8], Wk/Wv [512,64], Wq [8,512,64], Wo [8,64,512], biases.
  TOP_K=2 routed experts act as the two attention heads.

Sharding: core c = 2*b + h handles batch b, query-half h (1024 query tokens),
with the full 2048 keys/values of batch b.

Key structural choices (v3, on top of v2's algorithm):
  - bk dropped (softmax-invariant); bv folded into bo on the host.
  - DMA priority order w0a[Wg..bo] -> qhi -> w0b -> qlo -> w0a[Wk|Wv] ->
    kTn -> vTn, all dispatched from the Sync queue so ring order is kept:
    the router path (A2) and q_all/staging start as soon as their bytes
    land; A1 (khT2) and vh fill the PE while kTn/vTn stream in.
  - The PE stream is kept dense (bias preseeds, A2-hi/qall/A2-lo/A1/vh
    interleaved in DMA-arrival order) to ride the DVFS p-state up; idle
    gaps drop the PE from 2.4GHz to 1.2/0.65GHz for microseconds.
  - Top-2 router selection is fp32-exact via 3-term bf16 hi/lo logits.
  - Expert gather: one-hot mask (DVE stt) + e-segment reduce per head;
    engine-queue and PSUM-pool rotations tuned so the serial DVE chain,
    not pool back-pressure, is the only pre-C limit.
  - bo' bias rides the D-phase scatter: ds col 64 per head = denom*s =
    normalized gate, scattered to cm[512+e_h]; the output projection has
    a 5th accumulating matmul (cm[:,512:520]^T @ bo8) instead of a
    comb/combT path.
  - Phase C per kc: one [128,1024] 2-bank logits tile (both heads via
    row-grouped concurrent matmuls), one exp, 2 attention matmuls with a
    ones column for the softmax denominator, software-pipelined.
    B-blocks for qt4-7 interleave into C(half0), D-blocks for qt0-3 into
    C(half1) with D3 split into transpose/matmul sub-blocks; the drain
    fires all four D1/scatters first, then pipelines the D3s.
  - Output is written bf16; host casts to fp32.

Rejected on measurement: fp8e4 DoubleRow attention (2x PE) -- the ISA
only accepts DoubleRow at tile (128,64..128)/pos(0,0), so the softmax
denominator (65th output row) cannot ride the matmul, and every separate
denominator pass costs back the savings in moving-data cycles or PSUM
banks.  GpSimd offload of the gather: Pool rejects per-partition scalar
ops (TensorScalarPtr), tensor_reduce is C-axis only, InstPool is
DVE-only on trn2, and indirect_copy shares indices per 16-partition
group.
"""

import numpy as np

import concourse.bass as bass
import concourse.mybir as mybir
import concourse.tile as tile
from concourse import bacc
from concourse import bass_utils
from concourse.masks import make_identity

P = 128
D = 512          # d_model
T = 2048         # kv tokens per core (full batch)
NQ = 1024        # query tokens per core
E = 8            # experts
DK = 64          # head dim
DC = D // P      # 4 contraction chunks
NKC = T // P     # 16 key chunks
NQT = NQ // P    # 8 query tiles
VW = DK + 1      # vh columns + ones column (denominator trick)
HD = 512         # phase-C column granularity (half of NQ)
DSW = 2 * VW     # ds width: [at0*s0 | g0 | at1*s1 | g1] = 130
CMW = D + E      # cm width: (d e) block + 8 gate slots = 520

FP = mybir.dt.float32
U32 = mybir.dt.uint32
I16 = mybir.dt.int16
BF = mybir.dt.bfloat16
AF = mybir.ActivationFunctionType
OP = mybir.AluOpType
AX = mybir.AxisListType

# ---- w0a packed layout (bf16 columns), identical to v2 ----
_W0A = {}
_off = 0
for name, cols in [("Wk2s", DC * P), ("Wv", DC * DK), ("Wg_hi", DC * E),
                   ("Wg_lo", DC * E), ("ones", P), ("bq_row", D), ("bo", D)]:
    _W0A[name] = _off
    _off += cols
_W0A_COLS = _off
_W0B_COLS = 2 * DC * D   # Wq_f (e d) | Wo_f (e d)


def _emit(nc, tc, ctx):
    const = ctx.enter_context(tc.tile_pool(name="const", bufs=1))
    persist = ctx.enter_context(tc.tile_pool(name="persist", bufs=1))
    work = ctx.enter_context(tc.tile_pool(name="work", bufs=4))
    bsm = ctx.enter_context(tc.tile_pool(name="bsm", bufs=8))
    expp = ctx.enter_context(tc.tile_pool(name="expp", bufs=4))
    dpool = ctx.enter_context(tc.tile_pool(name="dpool", bufs=2))
    ps_log = ctx.enter_context(tc.tile_pool(name="ps_log", bufs=2, space="PSUM"))
    ps_att = ctx.enter_context(tc.tile_pool(name="ps_att", bufs=2, space="PSUM"))
    ps_m = ctx.enter_context(tc.tile_pool(name="ps_m", bufs=2, space="PSUM"))

    dram = {}
    for name, shape, dt in [
        ("w0a", [P, _W0A_COLS], BF), ("w0b", [P, _W0B_COLS], BF),
        ("kTn", [P, DC * T], BF), ("vTn", [P, DC * T], BF),
        ("qhi", [P, DC * NQ], BF), ("qlo", [P, DC * NQ], BF),
    ]:
        dram[name] = nc.dram_tensor(name, shape, dt, kind="ExternalInput").ap()
    out_d = nc.dram_tensor("out", [NQ, D], BF, kind="ExternalOutput").ap()

    # ---- DMA dispatch, priority order (single engine keeps ring order) ----
    w0a = const.tile([P, _W0A_COLS], BF, tag="w0a")
    qhi_t = persist.tile([P, DC * NQ], BF, tag="qhi")
    w0b = const.tile([P, _W0B_COLS], BF, tag="w0b")
    qlo_t = persist.tile([P, DC * NQ], BF, tag="qlo")
    kTt = persist.tile([P, DC * T], BF, tag="kTt")
    vTt = persist.tile([P, DC * T], BF, tag="vTt")
    _kv = _W0A["Wg_hi"]        # cols 0.._kv = Wk2s|Wv (A1/vh only)
    nc.sync.dma_start(w0a[:, _kv:], dram["w0a"][:, _kv:])
    nc.sync.dma_start(qhi_t[:], dram["qhi"])
    nc.sync.dma_start(w0b[:], dram["w0b"])
    nc.sync.dma_start(qlo_t[:], dram["qlo"])
    nc.sync.dma_start(w0a[:, :_kv], dram["w0a"][:, :_kv])
    for j in range(4):
        cs = slice(j * T, (j + 1) * T)
        nc.sync.dma_start(kTt[:, cs], dram["kTn"][:, cs])
    for j in range(4):
        cs = slice(j * T, (j + 1) * T)
        nc.sync.dma_start(vTt[:, cs], dram["vTn"][:, cs])

    def w0(name, r0, r1, c0, c1):
        base = _W0A[name]
        return w0a[r0:r1, base + c0:base + c1]

    Wk2s = {dc: w0("Wk2s", 0, P, dc * P, (dc + 1) * P) for dc in range(DC)}
    Wv = {dc: w0("Wv", 0, P, dc * DK, (dc + 1) * DK) for dc in range(DC)}
    Wg_hi = {dc: w0("Wg_hi", 0, P, dc * E, (dc + 1) * E) for dc in range(DC)}
    Wg_lo = {dc: w0("Wg_lo", 0, P, dc * E, (dc + 1) * E) for dc in range(DC)}
    ones_blk = w0("ones", 0, P, 0, P)
    ones_row = w0("ones", 0, 1, 0, P)        # [1,128] lhsT for bias matmul
    bq_row = w0("bq_row", 0, 1, 0, D)        # [1,512] (e d)... (d e) order
    bo8 = w0("bo", 0, E, 0, D)               # [8,512] bo' = bo + bv@Wo
    Wq_f = {dc: w0b[:, dc * D:(dc + 1) * D] for dc in range(DC)}
    Wo_f = {dc: w0b[:, DC * D + dc * D:DC * D + (dc + 1) * D] for dc in range(DC)}
    qT_hi = {dc: qhi_t[:, dc * NQ:(dc + 1) * NQ] for dc in range(DC)}
    qT_lo = {dc: qlo_t[:, dc * NQ:(dc + 1) * NQ] for dc in range(DC)}

    # ---- constants (gpsimd standard lib + DVE) ----
    ident = const.tile([P, P], FP, tag="ident")
    make_identity(nc, ident[:])
    ident_b = const.tile([P, P], BF, tag="ident_b")
    make_identity(nc, ident_b[:])
    iota65x8 = const.tile([P, VW], FP, tag="iota65x8")   # 0,8,...,504,512
    nc.gpsimd.iota(iota65x8[:], pattern=[[8, VW]], channel_multiplier=0,
                   allow_small_or_imprecise_dtypes=True)
    iota_e = const.tile([P, DK * E], BF, tag="iota_e")   # expert id, (d e) cols
    nc.gpsimd.iota(iota_e[:].rearrange("p (d e) -> p d e", e=E),
                   pattern=[[0, DK], [1, E]], channel_multiplier=0,
                   allow_small_or_imprecise_dtypes=True)

    # ---- persistent intermediates ----
    khT2 = persist.tile([P, T], BF, tag="khT2")
    vh_aug = persist.tile([P, NKC * VW], BF, tag="vh_aug")
    routerT = persist.tile([E, NQ], FP, tag="routerT")
    qselT2 = persist.tile([P, NQ], BF, tag="qselT2")
    attnT = [persist.tile([VW, NQ], BF, tag=f"attnT{h}", name=f"attnT{h}")
             for h in range(2)]
    g_all = persist.tile([P, 2 * NQT], FP, tag="g_all")    # gates, 2 per qt
    idx16 = persist.tile([P, DSW * NQT], I16, tag="idx16") # scatter idxs per qt

    # vh_aug ones columns (col 64 of each kc block)
    nc.vector.tensor_copy(
        vh_aug[:].rearrange("p (c w) -> p c w", w=VW)[:, :, DK],
        ones_blk[:, 0:NKC])

    # ================= emission =================
    # -- 1. bias preseed for qt0/qt1 (warms PE during DMA wait) --
    qa_ps = {}
    for qt in range(2):
        ps_qa = ps_att.tile([P, D], FP, tag="ps_a", name=f"ps_qa{qt}")
        nc.tensor.matmul(ps_qa[:], ones_row, bq_row, start=True, stop=False,
                         skip_group_check=True)
        qa_ps[qt] = ps_qa

    # -- 2. A2 hi terms (router logits from q_hi), groups stay open --
    ps_r = {}
    for half in range(2):
        ps_r[half] = ps_m.tile([E, HD], FP, tag="ps", name=f"ps_r{half}")
    for dc in range(DC):
        for half in range(2):
            hs = slice(half * HD, (half + 1) * HD)
            nc.tensor.matmul(ps_r[half][:], Wg_hi[dc][:], qT_hi[dc][:, hs],
                             start=(dc == 0), stop=False, skip_group_check=True)
            nc.tensor.matmul(ps_r[half][:], Wg_lo[dc][:], qT_hi[dc][:, hs],
                             start=False, stop=False, skip_group_check=True)

    # -- 3. q_all per qt (B0-3), staged to SBUF by ACT --
    qa_b = {}
    for qt in range(4):
        qs = slice(qt * P, (qt + 1) * P)
        if qt < 2:
            ps_qa = qa_ps[qt]
        else:
            ps_qa = ps_att.tile([P, D], FP, tag="ps_a", name=f"ps_qa{qt}")
            nc.tensor.matmul(ps_qa[:], ones_row, bq_row, start=True, stop=False,
                             skip_group_check=True)
        for dc in range(DC):
            nc.tensor.matmul(ps_qa[:], qT_hi[dc][:, qs], Wq_f[dc][:],
                             start=False, stop=(dc == DC - 1),
                             skip_group_check=True)
        qa = work.tile([P, D], BF, tag=f"qa{qt % 4}", name=f"qa{qt}")
        nc.scalar.activation(qa[:], ps_qa[:], AF.Copy)   # frees ps_att buf
        qa_b[qt] = qa

    # -- 4. A2 lo terms + routerT copies --
    for dc in range(DC):
        for half in range(2):
            hs = slice(half * HD, (half + 1) * HD)
            nc.tensor.matmul(ps_r[half][:], Wg_hi[dc][:], qT_lo[dc][:, hs],
                             start=False, stop=(dc == DC - 1),
                             skip_group_check=True)
    for half in range(2):
        hs = slice(half * HD, (half + 1) * HD)
        nc.scalar.activation(routerT[:, hs], ps_r[half][:], AF.Copy)

    # -- 5/6. A1 chunks interleaved with B top-k/gather chains --
    def emit_A1_chunk(j):
        cs = slice(j * HD, (j + 1) * HD)
        ps = ps_m.tile([P, HD], FP, tag="ps", name=f"ps_kh{j}")
        for dc in range(DC):
            nc.tensor.matmul(ps[:], Wk2s[dc][:], kTt[:, j * T + dc * HD:
                                                     j * T + (dc + 1) * HD],
                             start=(dc == 0), stop=(dc == DC - 1))
        nc.scalar.activation(khT2[:, cs], ps[:], AF.Copy)

    qsel2s = {}
    b_m8 = {}
    b_if8 = {}

    def emit_B_chain(qt, in_c0):
        qs = slice(qt * P, (qt + 1) * P)
        # PE: router logits transpose for this qt
        tp = ps_m if in_c0 else ps_att
        ps_lg = tp.tile([P, E], FP, tag="ps" if in_c0 else "ps_a",
                        name=f"ps_lg{qt}")
        nc.tensor.matmul(ps_lg[:], routerT[:, qs], ident[:E, :E],
                         is_transpose=True)
        # DVE: top-2 selection straight from PSUM
        m8 = bsm.tile([P, E], FP, tag="m8", name=f"m8_{qt}")
        b_m8[qt] = m8
        nc.vector.max(out=m8[:], in_=ps_lg[:])
        i8 = bsm.tile([P, E], U32, tag="i8", name=f"i8_{qt}")
        nc.vector.max_index(i8[:], m8[:], ps_lg[:])
        if8 = bsm.tile([P, 2], FP, tag="if8", name=f"if8_{qt}")
        nc.gpsimd.tensor_copy(if8[:], i8[:, 0:2])
        b_if8[qt] = if8
        # gather: DVE builds the one-hot-masked q_all; gp pools the e
        # segments (window-8 avg; the /8 is folded into the logit scale,
        # Wk2s is packed unscaled on the host)
        qa = qa_b[qt]
        qsel2 = bsm.tile([P, P], BF, tag="qsel2", name=f"qsel2_{qt}")
        for h in range(2):
            u = work.tile([P, D], BF, tag=f"u{h}", name=f"u{qt}_{h}")
            nc.vector.scalar_tensor_tensor(
                u[:], iota_e[:], if8[:, h:h + 1], qa[:],
                op0=OP.is_equal, op1=OP.mult)
            with nc.allow_low_precision(reason="one-hot masked sum: only "
                                        "one of the 8 summands is nonzero"):
                nc.vector.reduce_sum(qsel2[:, h * DK:(h + 1) * DK],
                                     u[:].rearrange("p (d e) -> p d e", e=E),
                                     axis=AX.X)
        qsel2s[qt] = qsel2

    def emit_B_idx(qt):
        if8 = b_if8[qt]
        for h in range(2):
            nc.vector.tensor_scalar(
                idx16[:, qt * DSW + h * VW:qt * DSW + (h + 1) * VW],
                iota65x8[:], if8[:, h:h + 1], None, op0=OP.add)

    def emit_B_gates(qt, in_c0):
        # gates (normalized): exp of sorted logits, row sum, scale.
        # Not on the C0 critical path (only D1 reads g_all).
        m8 = b_m8[qt]
        e8s = bsm.tile([P, E], FP, tag="e8s", name=f"e8s_{qt}")
        ssum = bsm.tile([P, 1], FP, tag="ssum", name=f"ssum_{qt}")
        if in_c0:
            nc.scalar.activation(e8s[:], m8[:], AF.Exp)
            with nc.allow_low_precision(reason="8-term fp32 sum into fp32"):
                nc.vector.reduce_sum(ssum[:], e8s[:], axis=AX.X)
        else:
            nc.scalar.activation(e8s[:], m8[:], AF.Exp, accum_out=ssum[:])
        srec = bsm.tile([P, 1], FP, tag="srec", name=f"srec_{qt}")
        nc.vector.reciprocal(srec[:], ssum[:])
        nc.vector.tensor_scalar(g_all[:, 2 * qt:2 * qt + 2], e8s[:, 0:2],
                                srec[:], None, op0=OP.mult)

    def emit_B_fin(qt):
        qs = slice(qt * P, (qt + 1) * P)
        qsel2 = qsel2s.pop(qt)
        ps_qsT = ps_m.tile([P, P], BF, tag="ps", name=f"ps_qsT{qt}")
        nc.tensor.matmul(ps_qsT[:], qsel2[:], ident_b[:], is_transpose=True)
        nc.vector.tensor_copy(qselT2[:, qs], ps_qsT[:])

    # -- 7. vh (key-major) one kc --
    def emit_vh_kc(kc, act_copy):
        ps = ps_m.tile([P, DK], FP, tag="ps", name=f"ps_vh{kc}")
        for dc in range(DC):
            nc.tensor.matmul(ps[:], vTt[:, kc * D + dc * P:kc * D + (dc + 1) * P],
                             Wv[dc][:], start=(dc == 0), stop=(dc == DC - 1))
        if act_copy:
            nc.scalar.activation(vh_aug[:, kc * VW:kc * VW + DK], ps[:], AF.Copy)
        else:
            nc.vector.tensor_copy(vh_aug[:, kc * VW:kc * VW + DK], ps[:])

    # A1 chunks interleaved with B chains (PE: kh c0,c1, lg0, lg1, kh c2,
    # c3, lg2, lg3 — keeps the in-order PE queue aligned with DMA arrival)
    emit_B_chain(0, in_c0=False)
    emit_A1_chunk(0)
    emit_B_chain(1, in_c0=False)
    emit_A1_chunk(1)
    emit_A1_chunk(2)
    emit_B_chain(2, in_c0=False)
    emit_A1_chunk(3)
    emit_B_chain(3, in_c0=False)

    # vh interleaved with the fin transposes
    for kc in range(4):
        emit_vh_kc(kc, act_copy=True)
    emit_B_fin(0)
    for kc in range(4, 8):
        emit_vh_kc(kc, act_copy=True)
    emit_B_fin(1)
    for kc in range(8, 12):
        emit_vh_kc(kc, act_copy=True)
    emit_B_fin(2)
    for kc in range(12, 16):
        emit_vh_kc(kc, act_copy=True)
    emit_B_fin(3)


    # -- 8. B block (PE part) for qt4-7, emitted inside C0 --
    def emit_B_mm(qt):
        qs = slice(qt * P, (qt + 1) * P)
        ps_qa = ps_m.tile([P, D], FP, tag="ps", name=f"ps_qa{qt}")
        nc.tensor.matmul(ps_qa[:], ones_row, bq_row, start=True, stop=False,
                         skip_group_check=True)
        for dc in range(DC):
            nc.tensor.matmul(ps_qa[:], qT_hi[dc][:, qs], Wq_f[dc][:],
                             start=False, stop=(dc == DC - 1),
                             skip_group_check=True)
        qa = work.tile([P, D], BF, tag=f"qa{qt % 4}", name=f"qa{qt}")
        nc.vector.tensor_copy(qa[:], ps_qa[:])   # DVE staging inside C0
        qa_b[qt] = qa

    # -- 9. D1 for one qt: transpose attn back, scale (incl. gate col),
    #       scatter to cm (gate-slot augmented) --
    cms = {}

    def emit_D1(qt):
        qs = slice(qt * P, (qt + 1) * P)
        h_at = []
        for h in range(2):
            ps_at = ps_m.tile([P, VW], BF, tag="ps", name=f"ps_at{qt % 2}_{h}")
            nc.tensor.matmul(ps_at[:], attnT[h][:, qs], ident_b[:VW, :VW],
                             is_transpose=True)
            at = dpool.tile([P, VW], FP, tag=f"at{qt % 2}_{h}",
                            name=f"at{qt % 2}_{h}")
            nc.vector.tensor_copy(at[:], ps_at[:])
            h_at.append(at)
        ds = dpool.tile([P, DSW], BF, tag=f"ds{qt % 2}", name=f"ds{qt % 2}")
        for h in range(2):
            dinv = bsm.tile([P, 1], FP, tag="dinv", name=f"dinv{qt}_{h}")
            nc.vector.reciprocal(dinv[:], h_at[h][:, DK:DK + 1])
            s = bsm.tile([P, 1], FP, tag="s", name=f"s{qt}_{h}")
            nc.vector.tensor_tensor(s[:], g_all[:, 2 * qt + h:2 * qt + h + 1],
                                    dinv[:], op=OP.mult)
            # cols 0..63: at*s ; col 64: denom*s = normalized gate
            nc.vector.tensor_scalar(ds[:, h * VW:(h + 1) * VW],
                                    h_at[h][:, 0:VW], s[:], None, op0=OP.mult)
        cm = dpool.tile([P, CMW], BF, tag=f"cm{qt % 2}", name=f"cm{qt % 2}")
        nc.gpsimd.local_scatter(cm[:], ds[:], idx16[:, qt * DSW:(qt + 1) * DSW],
                                channels=P, num_elems=CMW, num_idxs=DSW)
        cms[qt] = cm

    # -- 10. D3 for one qt: transpose cm, output projection (+gate-slot
    #        chunk against bo8), store --
    d3_cts = {}

    def emit_D3a(qt, late):
        cm = cms.pop(qt)
        tp = ps_log if late else ps_m
        cTs = []
        for ci in range(DC):
            ps_ct = tp.tile([P, P], BF, tag="ps_lg2" if late else "ps",
                            name=f"ps_ct{ci}")
            nc.tensor.matmul(ps_ct[:], cm[:, ci * P:(ci + 1) * P], ident_b[:],
                             is_transpose=True)
            cT = work.tile([P, P], BF, tag=f"cT{ci}", name=f"cT{ci}_{qt}")
            nc.vector.tensor_copy(cT[:], ps_ct[:])
            cTs.append(cT)
        ps_c5 = tp.tile([E, P], BF, tag="ps_lg2" if late else "ps",
                        name="ps_ct5")
        nc.tensor.matmul(ps_c5[:], cm[:, D:D + E], ident_b[:],
                         is_transpose=True)
        cT5 = work.tile([E, P], BF, tag="cT5", name=f"cT5_{qt}")
        nc.vector.tensor_copy(cT5[:], ps_c5[:])
        d3_cts[qt] = (cTs, cT5)

    def emit_D3b(qt, late):
        qs = slice(qt * P, (qt + 1) * P)
        cTs, cT5 = d3_cts.pop(qt)
        ps_o = ps_m.tile([P, D], FP, tag="ps", name=f"ps_o{qt}")
        for ci in range(DC):
            nc.tensor.matmul(ps_o[:], cTs[ci][:], Wo_f[ci][:],
                             start=(ci == 0), stop=False, skip_group_check=True)
        nc.tensor.matmul(ps_o[:], cT5[:], bo8, start=False, stop=True,
                         skip_group_check=True)
        o = work.tile([P, D], BF, tag="o", name=f"o{qt}")
        if late:
            nc.scalar.activation(o[:], ps_o[:], AF.Copy)
        else:
            nc.vector.tensor_copy(o[:], ps_o[:])
        nc.sync.dma_start(out_d[qs, :], o[:])

    # -- 11. phase C for one half, with interleaved extra blocks --
    def emit_C(half, extras):
        hs = slice(half * HD, (half + 1) * HD)
        ps_a = [ps_att.tile([VW, HD], FP, tag="ps_a", name=f"ps_a{h}")
                for h in range(2)]
        pending = None
        for kc in range(NKC):
            ps_lg2 = ps_log.tile([P, 2 * HD], FP, tag="ps_lg2")
            for h in range(2):
                rg = slice(h * DK, (h + 1) * DK)
                nc.tensor.matmul(ps_lg2[:, h * HD:(h + 1) * HD],
                                 khT2[rg, kc * P:(kc + 1) * P],
                                 qselT2[rg, hs], start=True, stop=True)
            if pending is not None:
                pkc, pex = pending
                for h in range(2):
                    nc.tensor.matmul(ps_a[h][:],
                                     vh_aug[:, pkc * VW:(pkc + 1) * VW],
                                     pex[:, h * HD:(h + 1) * HD],
                                     start=(pkc == 0), stop=(pkc == NKC - 1),
                                     skip_group_check=True)
            ex = expp.tile([P, 2 * HD], BF, tag="ex")
            nc.scalar.activation(ex[:], ps_lg2[:], AF.Exp)
            pending = (kc, ex)
            for fn in extras.get(kc, ()):
                fn()
        pkc, pex = pending
        for h in range(2):
            nc.tensor.matmul(ps_a[h][:], vh_aug[:, pkc * VW:(pkc + 1) * VW],
                             pex[:, h * HD:(h + 1) * HD],
                             start=(pkc == 0), stop=(pkc == NKC - 1),
                             skip_group_check=True)
        nc.vector.tensor_copy(attnT[0][:, hs], ps_a[0][:])
        nc.vector.tensor_copy(attnT[1][:, hs], ps_a[1][:])

    # B_mm(4,5) pre-C so their DVE staging runs before the C0 extras and
    # frees the ps_m rotation for the in-C0 chains
    emit_B_mm(4)
    emit_B_mm(5)
    for qt in range(4):
        emit_B_gates(qt, in_c0=False)
        emit_B_idx(qt)

    # C half 0, with B(4..7) interleaved into the matmul stream
    emit_C(0, {0: [lambda: emit_B_chain(4, True), lambda: emit_B_mm(6)],
               1: [lambda: emit_B_chain(5, True), lambda: emit_B_mm(7)],
               2: [lambda: emit_B_chain(6, True)],
               3: [lambda: emit_B_chain(7, True)],
               8: [lambda: emit_B_fin(4)],
               9: [lambda: emit_B_gates(4, True), lambda: emit_B_idx(4)],
               10: [lambda: emit_B_fin(5)],
               11: [lambda: emit_B_gates(5, True), lambda: emit_B_idx(5)],
               12: [lambda: emit_B_fin(6)],
               13: [lambda: emit_B_gates(6, True), lambda: emit_B_idx(6)],
               14: [lambda: emit_B_fin(7)],
               15: [lambda: emit_B_gates(7, True), lambda: emit_B_idx(7)]})

    # C half 1, with D(0..3) interleaved (their attnT half-0 data is ready)
    emit_C(1, {0: [lambda: emit_D1(0)],
               2: [lambda: emit_D1(1)],
               3: [lambda: emit_D3a(0, False)],
               4: [lambda: emit_D3b(0, False)],
               5: [lambda: emit_D1(2)],
               6: [lambda: emit_D3a(1, False)],
               7: [lambda: emit_D3b(1, False)],
               8: [lambda: emit_D1(3)],
               9: [lambda: emit_D3a(2, False)],
               10: [lambda: emit_D3b(2, False)],
               12: [lambda: emit_D3a(3, False)],
               13: [lambda: emit_D3b(3, False)]})

    # drain: scatters first (gpsimd runs ahead), D3s pipelined behind
    emit_D1(4)
    emit_D1(5)
    emit_D1(6)
    emit_D1(7)
    emit_D3a(4, True)
    emit_D3b(4, True)
    emit_D3a(5, True)
    emit_D3b(5, True)
    emit_D3a(6, True)
    emit_D3b(6, True)
    emit_D3a(7, True)
    emit_D3b(7, True)


_PROGRAM = None


def get_program():
    global _PROGRAM
    if _PROGRAM is None:
        nc = bacc.Bacc("TRN2", target_bir_lowering=False, debug=False,
                       enable_asserts=False, num_devices=8)
        from contextlib import ExitStack
        with tile.TileContext(nc) as tc, ExitStack() as ctx:
            _emit(nc, tc, ctx)
        nc.compile()
        _PROGRAM = nc
    return _PROGRAM


def make_in_maps(query, key, value, Wg, Wk, bk, Wv, bv, Wq, bq, Wo, bo):
    import ml_dtypes
    BFNP = ml_dtypes.bfloat16

    def hilo(x):
        x = np.asarray(x, np.float32)
        hi = x.astype(BFNP)
        lo = (x - hi.astype(np.float32)).astype(BFNP)
        return hi, lo

    Wg_hi, Wg_lo = hilo(Wg)
    # kh pre-scaled by 1/8 == 1/sqrt(DK); doubled for the two head row-groups
    Wk2s = np.concatenate([np.asarray(Wk), np.asarray(Wk)], axis=1) * 0.125
    # bk shifts all logits of a query equally -> softmax-invariant: dropped.
    # (d e) ordering (col/row index = d*E + e)
    Wq_f = np.asarray(Wq).transpose(1, 2, 0).reshape(D, DK * E)
    Wo_f = np.asarray(Wo).transpose(1, 0, 2).reshape(DK * E, D)
    bq_f = np.asarray(bq).T.reshape(DK * E)
    # bv folds into bo since attention weights sum to 1
    bo_p = np.asarray(bo) + np.einsum('d,edm->em', np.asarray(bv, np.float32),
                                      np.asarray(Wo, np.float32))

    w0a = np.zeros((P, _W0A_COLS), BFNP)
    def put(name, rows, arr):
        base = _W0A[name]
        arr = np.asarray(arr, BFNP)
        w0a[rows, base:base + arr.shape[-1]] = arr
    for dc in range(DC):
        w0a[:, _W0A["Wk2s"] + dc * P:_W0A["Wk2s"] + (dc + 1) * P] = \
            np.asarray(Wk2s[dc * P:(dc + 1) * P, :], BFNP)
        w0a[:, _W0A["Wv"] + dc * DK:_W0A["Wv"] + (dc + 1) * DK] = \
            np.asarray(Wv, np.float32)[dc * P:(dc + 1) * P, :].astype(BFNP)
        w0a[:, _W0A["Wg_hi"] + dc * E:_W0A["Wg_hi"] + (dc + 1) * E] = \
            Wg_hi[dc * P:(dc + 1) * P, :]
        w0a[:, _W0A["Wg_lo"] + dc * E:_W0A["Wg_lo"] + (dc + 1) * E] = \
            Wg_lo[dc * P:(dc + 1) * P, :]
    w0a[:, _W0A["ones"]:_W0A["ones"] + P] = np.ones((P, P), BFNP)
    put("bq_row", 0, bq_f)
    put("bo", slice(0, E), bo_p)

    w0b = np.zeros((P, _W0B_COLS), BFNP)
    for dc in range(DC):
        w0b[:, dc * D:(dc + 1) * D] = \
            np.asarray(Wq_f, np.float32)[dc * P:(dc + 1) * P, :].astype(BFNP)
        w0b[:, DC * D + dc * D:DC * D + (dc + 1) * D] = \
            np.asarray(Wo_f, np.float32)[dc * P:(dc + 1) * P, :].astype(BFNP)

    def pack_chunks(x, n):  # [512, N] -> [128, 4*N] (dc-major columns)
        out = np.empty((P, DC * n), x.dtype)
        for dc in range(DC):
            out[:, dc * n:(dc + 1) * n] = x[dc * P:(dc + 1) * P, :]
        return np.ascontiguousarray(out)

    def pack_blocks(x, nblk, blkcols):  # [512, N] -> [128, nblk*(4*blkcols)]
        # block j = [dc0 cols | dc1 cols | dc2 cols | dc3 cols]
        out = np.empty((P, DC * nblk * blkcols), x.dtype)
        for j in range(nblk):
            for dc in range(DC):
                off = j * DC * blkcols + dc * blkcols
                out[:, off:off + blkcols] = \
                    x[dc * P:(dc + 1) * P, j * blkcols:(j + 1) * blkcols]
        return np.ascontiguousarray(out)

    shared = {"w0a": np.ascontiguousarray(w0a), "w0b": np.ascontiguousarray(w0b)}
    in_maps = []
    for b in range(4):
        kT = np.asarray(key[b], np.float32).T.astype(BFNP)     # [512, 2048]
        vT = np.asarray(value[b], np.float32).T.astype(BFNP)
        kTn = pack_blocks(kT, 4, HD)      # A1-chunk-major blocks of 512 keys
        vTn = pack_blocks(vT, NKC, P)     # kc-major blocks of 128 keys
        for h in range(2):
            qhi, qlo = hilo(np.asarray(query[b][h * NQ:(h + 1) * NQ, :]).T)
            in_maps.append({"kTn": kTn, "vTn": vTn,
                            "qhi": pack_chunks(qhi, NQ),
                            "qlo": pack_chunks(qlo, NQ), **shared})
    return in_maps


def kernel(query, key, value, Wg, Wk, bk, Wv, bv, Wq, bq, Wo, bo):
    in_maps = make_in_maps(query, key, value, Wg, Wk, bk, Wv, bv, Wq, bq, Wo, bo)
    nc = get_program()
    res = bass_utils.run_bass_kernel_spmd(nc, in_maps, core_ids=list(range(8)))
    outs = [np.asarray(res.results[c]["out"], np.float32) for c in range(8)]
    return np.concatenate(outs, axis=0).reshape(4, T, D)


# revision 41
# speedup vs baseline: 1.1470x; 1.1470x over previous
"""MoE-routed multi-head attention kernel for 8 Trainium2 NeuronCores. v3

Problem shape (hardcoded):
  query/key/value: [4, 2048, 512] f32
  Wg [512,8], Wk/Wv [512,64], Wq [8,512,64], Wo [8,64,512], biases.
  TOP_K=2 routed experts act as the two attention heads.

Sharding: core c = 2*b + h handles batch b, query-half h (1024 query tokens),
with the full 2048 keys/values of batch b.

Key structural choices (v3, on top of v2's algorithm):
  - bk dropped (softmax-invariant); bv folded into bo on the host.
  - DMA priority order w0a[Wg..bo] -> qhi -> w0b -> qlo -> w0a[Wk|Wv] ->
    kTn -> vTn, all dispatched from the Sync queue so ring order is kept:
    the router path (A2) and q_all/staging start as soon as their bytes
    land; A1 (khT2) and vh fill the PE while kTn/vTn stream in.
  - The PE stream is kept dense (bias preseeds, A2-hi/qall/A2-lo/A1/vh
    interleaved in DMA-arrival order) to ride the DVFS p-state up; idle
    gaps drop the PE from 2.4GHz to 1.2/0.65GHz for microseconds.
  - Top-2 router selection is fp32-exact via 3-term bf16 hi/lo logits.
  - Expert gather: one-hot mask (DVE stt) + e-segment reduce per head;
    engine-queue and PSUM-pool rotations tuned so the serial DVE chain,
    not pool back-pressure, is the only pre-C limit.
  - bo' bias rides the D-phase scatter: ds col 64 per head = denom*s =
    normalized gate, scattered to cm[512+e_h]; the output projection has
    a 5th accumulating matmul (cm[:,512:520]^T @ bo8) instead of a
    comb/combT path.
  - Phase C per kc: one [128,1024] 2-bank logits tile (both heads via
    row-grouped concurrent matmuls), one exp, 2 attention matmuls with a
    ones column for the softmax denominator, software-pipelined.
    B-blocks for qt4-7 interleave into C(half0), D-blocks for qt0-3 into
    C(half1) with D3 split into transpose/matmul sub-blocks; the drain
    fires all four D1/scatters first, then pipelines the D3s.
  - Output is written bf16; host casts to fp32.

Rejected on measurement: fp8e4 DoubleRow attention (2x PE) -- the ISA
only accepts DoubleRow at tile (128,64..128)/pos(0,0), so the softmax
denominator (65th output row) cannot ride the matmul, and every separate
denominator pass costs back the savings in moving-data cycles or PSUM
banks.  GpSimd offload of the gather: Pool rejects per-partition scalar
ops (TensorScalarPtr), tensor_reduce is C-axis only, InstPool is
DVE-only on trn2, and indirect_copy shares indices per 16-partition
group.
"""

import numpy as np

import concourse.bass as bass
import concourse.mybir as mybir
import concourse.tile as tile
from concourse import bacc
from concourse import bass_utils
from concourse.masks import make_identity

P = 128
D = 512          # d_model
T = 2048         # kv tokens per core (full batch)
NQ = 1024        # query tokens per core
E = 8            # experts
DK = 64          # head dim
DC = D // P      # 4 contraction chunks
NKC = T // P     # 16 key chunks
NQT = NQ // P    # 8 query tiles
VW = DK + 1      # vh columns + ones column (denominator trick)
HD = 512         # phase-C column granularity (half of NQ)
DSW = 2 * VW     # ds width: [at0*s0 | g0 | at1*s1 | g1] = 130
CMW = D + E      # cm width: (d e) block + 8 gate slots = 520

FP = mybir.dt.float32
U32 = mybir.dt.uint32
I16 = mybir.dt.int16
BF = mybir.dt.bfloat16
AF = mybir.ActivationFunctionType
OP = mybir.AluOpType
AX = mybir.AxisListType

# ---- w0a packed layout (bf16 columns), identical to v2 ----
_W0A = {}
_off = 0
for name, cols in [("Wk2s", DC * P), ("Wv", DC * DK), ("Wg_hi", DC * E),
                   ("Wg_lo", DC * E), ("ones", P), ("bq_row", D), ("bo", D)]:
    _W0A[name] = _off
    _off += cols
_W0A_COLS = _off
_W0B_COLS = 2 * DC * D   # Wq_f (e d) | Wo_f (e d)


def _emit(nc, tc, ctx):
    const = ctx.enter_context(tc.tile_pool(name="const", bufs=1))
    persist = ctx.enter_context(tc.tile_pool(name="persist", bufs=1))
    work = ctx.enter_context(tc.tile_pool(name="work", bufs=4))
    bsm = ctx.enter_context(tc.tile_pool(name="bsm", bufs=8))
    expp = ctx.enter_context(tc.tile_pool(name="expp", bufs=4))
    dpool = ctx.enter_context(tc.tile_pool(name="dpool", bufs=2))
    ps_log = ctx.enter_context(tc.tile_pool(name="ps_log", bufs=2, space="PSUM"))
    ps_att = ctx.enter_context(tc.tile_pool(name="ps_att", bufs=2, space="PSUM"))
    ps_m = ctx.enter_context(tc.tile_pool(name="ps_m", bufs=2, space="PSUM"))

    dram = {}
    for name, shape, dt in [
        ("w0a", [P, _W0A_COLS], BF), ("w0b", [P, _W0B_COLS], BF),
        ("kTn", [P, DC * T], BF), ("vTn", [P, DC * T], BF),
        ("qhi", [P, DC * NQ], BF), ("qlo", [P, DC * NQ], BF),
    ]:
        dram[name] = nc.dram_tensor(name, shape, dt, kind="ExternalInput").ap()
    out_d = nc.dram_tensor("out", [NQ, D], BF, kind="ExternalOutput").ap()

    # ---- DMA dispatch, priority order (single engine keeps ring order) ----
    w0a = const.tile([P, _W0A_COLS], BF, tag="w0a")
    qhi_t = persist.tile([P, DC * NQ], BF, tag="qhi")
    w0b = const.tile([P, _W0B_COLS], BF, tag="w0b")
    qlo_t = persist.tile([P, DC * NQ], BF, tag="qlo")
    kTt = persist.tile([P, DC * T], BF, tag="kTt")
    vTt = persist.tile([P, DC * T], BF, tag="vTt")
    _kv = _W0A["Wg_hi"]        # cols 0.._kv = Wk2s|Wv (A1/vh only)
    nc.sync.dma_start(w0a[:, _kv:], dram["w0a"][:, _kv:])
    nc.sync.dma_start(qhi_t[:], dram["qhi"])
    nc.sync.dma_start(w0b[:], dram["w0b"])
    nc.sync.dma_start(qlo_t[:], dram["qlo"])
    nc.sync.dma_start(w0a[:, :_kv], dram["w0a"][:, :_kv])
    for j in range(4):
        cs = slice(j * T, (j + 1) * T)
        nc.sync.dma_start(kTt[:, cs], dram["kTn"][:, cs])
    for j in range(4):
        cs = slice(j * T, (j + 1) * T)
        nc.sync.dma_start(vTt[:, cs], dram["vTn"][:, cs])

    def w0(name, r0, r1, c0, c1):
        base = _W0A[name]
        return w0a[r0:r1, base + c0:base + c1]

    Wk2s = {dc: w0("Wk2s", 0, P, dc * P, (dc + 1) * P) for dc in range(DC)}
    Wv = {dc: w0("Wv", 0, P, dc * DK, (dc + 1) * DK) for dc in range(DC)}
    Wg_hi = {dc: w0("Wg_hi", 0, P, dc * E, (dc + 1) * E) for dc in range(DC)}
    Wg_lo = {dc: w0("Wg_lo", 0, P, dc * E, (dc + 1) * E) for dc in range(DC)}
    ones_blk = w0("ones", 0, P, 0, P)
    ones_row = w0("ones", 0, 1, 0, P)        # [1,128] lhsT for bias matmul
    bq_row = w0("bq_row", 0, 1, 0, D)        # [1,512] (e d)... (d e) order
    bo8 = w0("bo", 0, E, 0, D)               # [8,512] bo' = bo + bv@Wo
    Wq_f = {dc: w0b[:, dc * D:(dc + 1) * D] for dc in range(DC)}
    Wo_f = {dc: w0b[:, DC * D + dc * D:DC * D + (dc + 1) * D] for dc in range(DC)}
    qT_hi = {dc: qhi_t[:, dc * NQ:(dc + 1) * NQ] for dc in range(DC)}
    qT_lo = {dc: qlo_t[:, dc * NQ:(dc + 1) * NQ] for dc in range(DC)}

    # ---- constants (gpsimd standard lib + DVE) ----
    ident = const.tile([P, P], FP, tag="ident")
    make_identity(nc, ident[:])
    ident_b = const.tile([P, P], BF, tag="ident_b")
    make_identity(nc, ident_b[:])
    iota65x8 = const.tile([P, VW], FP, tag="iota65x8")   # 0,8,...,504,512
    nc.gpsimd.iota(iota65x8[:], pattern=[[8, VW]], channel_multiplier=0,
                   allow_small_or_imprecise_dtypes=True)
    iota_e = const.tile([P, DK * E], BF, tag="iota_e")   # expert id, (d e) cols
    nc.gpsimd.iota(iota_e[:].rearrange("p (d e) -> p d e", e=E),
                   pattern=[[0, DK], [1, E]], channel_multiplier=0,
                   allow_small_or_imprecise_dtypes=True)

    # ---- persistent intermediates ----
    khT2 = persist.tile([P, T], BF, tag="khT2")
    vh_aug = persist.tile([P, NKC * VW], BF, tag="vh_aug")
    routerT = persist.tile([E, NQ], FP, tag="routerT")
    qselT2 = persist.tile([P, NQ], BF, tag="qselT2")
    attnT = [persist.tile([VW, NQ], BF, tag=f"attnT{h}", name=f"attnT{h}")
             for h in range(2)]
    g_all = persist.tile([P, 2 * NQT], FP, tag="g_all")    # gates, 2 per qt
    idx16 = persist.tile([P, DSW * NQT], I16, tag="idx16") # scatter idxs per qt

    # vh_aug ones columns (col 64 of each kc block)
    nc.vector.tensor_copy(
        vh_aug[:].rearrange("p (c w) -> p c w", w=VW)[:, :, DK],
        ones_blk[:, 0:NKC])

    # ================= emission =================
    # -- 1. bias preseed for qt0/qt1 (warms PE during DMA wait) --
    qa_ps = {}
    for qt in range(2):
        ps_qa = ps_att.tile([P, D], FP, tag="ps_a", name=f"ps_qa{qt}")
        nc.tensor.matmul(ps_qa[:], ones_row, bq_row, start=True, stop=False,
                         skip_group_check=True)
        qa_ps[qt] = ps_qa

    # -- 2. A2 hi terms (router logits from q_hi), groups stay open --
    ps_r = {}
    for half in range(2):
        ps_r[half] = ps_m.tile([E, HD], FP, tag="ps", name=f"ps_r{half}")
    for dc in range(DC):
        for half in range(2):
            hs = slice(half * HD, (half + 1) * HD)
            nc.tensor.matmul(ps_r[half][:], Wg_hi[dc][:], qT_hi[dc][:, hs],
                             start=(dc == 0), stop=False, skip_group_check=True)
            nc.tensor.matmul(ps_r[half][:], Wg_lo[dc][:], qT_hi[dc][:, hs],
                             start=False, stop=False, skip_group_check=True)

    # -- 3. q_all per qt (B0-3), staged to SBUF by ACT --
    qa_b = {}
    for qt in range(4):
        qs = slice(qt * P, (qt + 1) * P)
        if qt < 2:
            ps_qa = qa_ps[qt]
        else:
            ps_qa = ps_att.tile([P, D], FP, tag="ps_a", name=f"ps_qa{qt}")
            nc.tensor.matmul(ps_qa[:], ones_row, bq_row, start=True, stop=False,
                             skip_group_check=True)
        for dc in range(DC):
            nc.tensor.matmul(ps_qa[:], qT_hi[dc][:, qs], Wq_f[dc][:],
                             start=False, stop=(dc == DC - 1),
                             skip_group_check=True)
        qa = work.tile([P, D], BF, tag=f"qa{qt % 4}", name=f"qa{qt}")
        nc.scalar.activation(qa[:], ps_qa[:], AF.Copy)   # frees ps_att buf
        qa_b[qt] = qa

    # -- 4. A2 lo terms + routerT copies --
    for dc in range(DC):
        for half in range(2):
            hs = slice(half * HD, (half + 1) * HD)
            nc.tensor.matmul(ps_r[half][:], Wg_hi[dc][:], qT_lo[dc][:, hs],
                             start=False, stop=(dc == DC - 1),
                             skip_group_check=True)
    for half in range(2):
        hs = slice(half * HD, (half + 1) * HD)
        nc.scalar.activation(routerT[:, hs], ps_r[half][:], AF.Copy)

    # -- 5/6. A1 chunks interleaved with B top-k/gather chains --
    def emit_A1_chunk(j):
        cs = slice(j * HD, (j + 1) * HD)
        ps = ps_m.tile([P, HD], FP, tag="ps", name=f"ps_kh{j}")
        for dc in range(DC):
            nc.tensor.matmul(ps[:], Wk2s[dc][:], kTt[:, j * T + dc * HD:
                                                     j * T + (dc + 1) * HD],
                             start=(dc == 0), stop=(dc == DC - 1))
        nc.scalar.activation(khT2[:, cs], ps[:], AF.Copy)

    qsel2s = {}
    b_m8 = {}
    b_if8 = {}

    def emit_B_chain(qt, in_c0):
        qs = slice(qt * P, (qt + 1) * P)
        # PE: router logits transpose for this qt
        tp = ps_m if in_c0 else ps_att
        ps_lg = tp.tile([P, E], FP, tag="ps" if in_c0 else "ps_a",
                        name=f"ps_lg{qt}")
        nc.tensor.matmul(ps_lg[:], routerT[:, qs], ident[:E, :E],
                         is_transpose=True)
        # DVE: top-2 selection straight from PSUM
        m8 = bsm.tile([P, E], FP, tag="m8", name=f"m8_{qt}")
        b_m8[qt] = m8
        nc.vector.max(out=m8[:], in_=ps_lg[:])
        i8 = bsm.tile([P, E], U32, tag="i8", name=f"i8_{qt}")
        nc.vector.max_index(i8[:], m8[:], ps_lg[:])
        if8 = bsm.tile([P, 2], FP, tag="if8", name=f"if8_{qt}")
        nc.gpsimd.tensor_copy(if8[:], i8[:, 0:2])
        b_if8[qt] = if8
        # gather: DVE builds the one-hot-masked q_all; gp pools the e
        # segments (window-8 avg; the /8 is folded into the logit scale,
        # Wk2s is packed unscaled on the host)
        qa = qa_b[qt]
        qsel2 = bsm.tile([P, P], BF, tag="qsel2", name=f"qsel2_{qt}")
        for h in range(2):
            u = work.tile([P, D], BF, tag=f"u{h}", name=f"u{qt}_{h}")
            nc.vector.scalar_tensor_tensor(
                u[:], iota_e[:], if8[:, h:h + 1], qa[:],
                op0=OP.is_equal, op1=OP.mult)
            with nc.allow_low_precision(reason="one-hot masked sum: only "
                                        "one of the 8 summands is nonzero"):
                nc.vector.reduce_sum(qsel2[:, h * DK:(h + 1) * DK],
                                     u[:].rearrange("p (d e) -> p d e", e=E),
                                     axis=AX.X)
        qsel2s[qt] = qsel2

    def emit_B_idx(qt):
        if8 = b_if8[qt]
        for h in range(2):
            nc.vector.tensor_scalar(
                idx16[:, qt * DSW + h * VW:qt * DSW + (h + 1) * VW],
                iota65x8[:], if8[:, h:h + 1], None, op0=OP.add)

    def emit_B_gates(qt, in_c0):
        # gates (normalized): exp of sorted logits, row sum, scale.
        # Not on the C0 critical path (only D1 reads g_all).
        m8 = b_m8[qt]
        e8s = bsm.tile([P, E], FP, tag="e8s", name=f"e8s_{qt}")
        ssum = bsm.tile([P, 1], FP, tag="ssum", name=f"ssum_{qt}")
        if in_c0:
            nc.scalar.activation(e8s[:], m8[:], AF.Exp)
            with nc.allow_low_precision(reason="8-term fp32 sum into fp32"):
                nc.vector.reduce_sum(ssum[:], e8s[:], axis=AX.X)
        else:
            nc.scalar.activation(e8s[:], m8[:], AF.Exp, accum_out=ssum[:])
        srec = bsm.tile([P, 1], FP, tag="srec", name=f"srec_{qt}")
        nc.vector.reciprocal(srec[:], ssum[:])
        nc.vector.tensor_scalar(g_all[:, 2 * qt:2 * qt + 2], e8s[:, 0:2],
                                srec[:], None, op0=OP.mult)

    def emit_B_fin(qt):
        qs = slice(qt * P, (qt + 1) * P)
        qsel2 = qsel2s.pop(qt)
        ps_qsT = ps_m.tile([P, P], BF, tag="ps", name=f"ps_qsT{qt}")
        nc.tensor.matmul(ps_qsT[:], qsel2[:], ident_b[:], is_transpose=True)
        nc.vector.tensor_copy(qselT2[:, qs], ps_qsT[:])

    # -- 7. vh (key-major) one kc --
    def emit_vh_kc(kc, act_copy):
        ps = ps_m.tile([P, DK], FP, tag="ps", name=f"ps_vh{kc}")
        for dc in range(DC):
            nc.tensor.matmul(ps[:], vTt[:, kc * D + dc * P:kc * D + (dc + 1) * P],
                             Wv[dc][:], start=(dc == 0), stop=(dc == DC - 1))
        if act_copy:
            nc.scalar.activation(vh_aug[:, kc * VW:kc * VW + DK], ps[:], AF.Copy)
        else:
            nc.vector.tensor_copy(vh_aug[:, kc * VW:kc * VW + DK], ps[:])

    # A1 chunks interleaved with B chains (PE: kh c0,c1, lg0, lg1, kh c2,
    # c3, lg2, lg3 — keeps the in-order PE queue aligned with DMA arrival)
    emit_A1_chunk(0)
    emit_A1_chunk(1)
    emit_B_chain(0, in_c0=False)
    emit_B_chain(1, in_c0=False)
    emit_A1_chunk(2)
    emit_A1_chunk(3)
    emit_B_chain(2, in_c0=False)
    emit_B_chain(3, in_c0=False)

    # vh interleaved with the fin transposes
    for kc in range(4):
        emit_vh_kc(kc, act_copy=True)
    emit_B_fin(0)
    for kc in range(4, 8):
        emit_vh_kc(kc, act_copy=True)
    emit_B_fin(1)
    for kc in range(8, 12):
        emit_vh_kc(kc, act_copy=True)
    emit_B_fin(2)
    for kc in range(12, 16):
        emit_vh_kc(kc, act_copy=True)
    emit_B_fin(3)


    # -- 8. B block (PE part) for qt4-7, emitted inside C0 --
    def emit_B_mm(qt):
        qs = slice(qt * P, (qt + 1) * P)
        ps_qa = ps_m.tile([P, D], FP, tag="ps", name=f"ps_qa{qt}")
        nc.tensor.matmul(ps_qa[:], ones_row, bq_row, start=True, stop=False,
                         skip_group_check=True)
        for dc in range(DC):
            nc.tensor.matmul(ps_qa[:], qT_hi[dc][:, qs], Wq_f[dc][:],
                             start=False, stop=(dc == DC - 1),
                             skip_group_check=True)
        qa = work.tile([P, D], BF, tag=f"qa{qt % 4}", name=f"qa{qt}")
        nc.vector.tensor_copy(qa[:], ps_qa[:])   # DVE staging inside C0
        qa_b[qt] = qa

    # -- 9. D1 for one qt: transpose attn back, scale (incl. gate col),
    #       scatter to cm (gate-slot augmented) --
    cms = {}

    def emit_D1(qt):
        qs = slice(qt * P, (qt + 1) * P)
        h_at = []
        for h in range(2):
            ps_at = ps_m.tile([P, VW], BF, tag="ps", name=f"ps_at{qt % 2}_{h}")
            nc.tensor.matmul(ps_at[:], attnT[h][:, qs], ident_b[:VW, :VW],
                             is_transpose=True)
            at = dpool.tile([P, VW], FP, tag=f"at{qt % 2}_{h}",
                            name=f"at{qt % 2}_{h}")
            nc.vector.tensor_copy(at[:], ps_at[:])
            h_at.append(at)
        ds = dpool.tile([P, DSW], BF, tag=f"ds{qt % 2}", name=f"ds{qt % 2}")
        for h in range(2):
            dinv = bsm.tile([P, 1], FP, tag="dinv", name=f"dinv{qt}_{h}")
            nc.vector.reciprocal(dinv[:], h_at[h][:, DK:DK + 1])
            s = bsm.tile([P, 1], FP, tag="s", name=f"s{qt}_{h}")
            nc.vector.tensor_tensor(s[:], g_all[:, 2 * qt + h:2 * qt + h + 1],
                                    dinv[:], op=OP.mult)
            # cols 0..63: at*s ; col 64: denom*s = normalized gate
            nc.vector.tensor_scalar(ds[:, h * VW:(h + 1) * VW],
                                    h_at[h][:, 0:VW], s[:], None, op0=OP.mult)
        cm = dpool.tile([P, CMW], BF, tag=f"cm{qt % 2}", name=f"cm{qt % 2}")
        nc.gpsimd.local_scatter(cm[:], ds[:], idx16[:, qt * DSW:(qt + 1) * DSW],
                                channels=P, num_elems=CMW, num_idxs=DSW)
        cms[qt] = cm

    # -- 10. D3 for one qt: transpose cm, output projection (+gate-slot
    #        chunk against bo8), store --
    d3_cts = {}

    def emit_D3a(qt, late):
        cm = cms.pop(qt)
        tp = ps_log if late else ps_m
        cTs = []
        for ci in range(DC):
            ps_ct = tp.tile([P, P], BF, tag="ps_lg2" if late else "ps",
                            name=f"ps_ct{ci}")
            nc.tensor.matmul(ps_ct[:], cm[:, ci * P:(ci + 1) * P], ident_b[:],
                             is_transpose=True)
            cT = work.tile([P, P], BF, tag=f"cT{ci}", name=f"cT{ci}_{qt}")
            nc.vector.tensor_copy(cT[:], ps_ct[:])
            cTs.append(cT)
        ps_c5 = tp.tile([E, P], BF, tag="ps_lg2" if late else "ps",
                        name="ps_ct5")
        nc.tensor.matmul(ps_c5[:], cm[:, D:D + E], ident_b[:],
                         is_transpose=True)
        cT5 = work.tile([E, P], BF, tag="cT5", name=f"cT5_{qt}")
        nc.vector.tensor_copy(cT5[:], ps_c5[:])
        d3_cts[qt] = (cTs, cT5)

    def emit_D3b(qt, late):
        qs = slice(qt * P, (qt + 1) * P)
        cTs, cT5 = d3_cts.pop(qt)
        ps_o = ps_m.tile([P, D], FP, tag="ps", name=f"ps_o{qt}")
        for ci in range(DC):
            nc.tensor.matmul(ps_o[:], cTs[ci][:], Wo_f[ci][:],
                             start=(ci == 0), stop=False, skip_group_check=True)
        nc.tensor.matmul(ps_o[:], cT5[:], bo8, start=False, stop=True,
                         skip_group_check=True)
        o = work.tile([P, D], BF, tag="o", name=f"o{qt}")
        if late:
            nc.scalar.activation(o[:], ps_o[:], AF.Copy)
        else:
            nc.vector.tensor_copy(o[:], ps_o[:])
        nc.sync.dma_start(out_d[qs, :], o[:])

    # -- 11. phase C for one half, with interleaved extra blocks --
    def emit_C(half, extras):
        hs = slice(half * HD, (half + 1) * HD)
        ps_a = [ps_att.tile([VW, HD], FP, tag="ps_a", name=f"ps_a{h}")
                for h in range(2)]
        pending = None
        for kc in range(NKC):
            ps_lg2 = ps_log.tile([P, 2 * HD], FP, tag="ps_lg2")
            for h in range(2):
                rg = slice(h * DK, (h + 1) * DK)
                nc.tensor.matmul(ps_lg2[:, h * HD:(h + 1) * HD],
                                 khT2[rg, kc * P:(kc + 1) * P],
                                 qselT2[rg, hs], start=True, stop=True)
            if pending is not None:
                pkc, pex = pending
                for h in range(2):
                    nc.tensor.matmul(ps_a[h][:],
                                     vh_aug[:, pkc * VW:(pkc + 1) * VW],
                                     pex[:, h * HD:(h + 1) * HD],
                                     start=(pkc == 0), stop=(pkc == NKC - 1),
                                     skip_group_check=True)
            ex = expp.tile([P, 2 * HD], BF, tag="ex")
            nc.scalar.activation(ex[:], ps_lg2[:], AF.Exp)
            pending = (kc, ex)
            for fn in extras.get(kc, ()):
                fn()
        pkc, pex = pending
        for h in range(2):
            nc.tensor.matmul(ps_a[h][:], vh_aug[:, pkc * VW:(pkc + 1) * VW],
                             pex[:, h * HD:(h + 1) * HD],
                             start=(pkc == 0), stop=(pkc == NKC - 1),
                             skip_group_check=True)
        nc.vector.tensor_copy(attnT[0][:, hs], ps_a[0][:])
        nc.vector.tensor_copy(attnT[1][:, hs], ps_a[1][:])

    # B_mm(4,5) pre-C so their DVE staging runs before the C0 extras and
    # frees the ps_m rotation for the in-C0 chains
    emit_B_mm(4)
    emit_B_mm(5)
    for qt in range(4):
        emit_B_gates(qt, in_c0=False)
        emit_B_idx(qt)

    # C half 0, with B(4..7) interleaved into the matmul stream
    emit_C(0, {0: [lambda: emit_B_chain(4, True), lambda: emit_B_mm(6)],
               1: [lambda: emit_B_chain(5, True), lambda: emit_B_mm(7)],
               2: [lambda: emit_B_chain(6, True)],
               3: [lambda: emit_B_chain(7, True)],
               8: [lambda: emit_B_fin(4)],
               9: [lambda: emit_B_gates(4, True), lambda: emit_B_idx(4)],
               10: [lambda: emit_B_fin(5)],
               11: [lambda: emit_B_gates(5, True), lambda: emit_B_idx(5)],
               12: [lambda: emit_B_fin(6)],
               13: [lambda: emit_B_gates(6, True), lambda: emit_B_idx(6)],
               14: [lambda: emit_B_fin(7)],
               15: [lambda: emit_B_gates(7, True), lambda: emit_B_idx(7)]})

    # C half 1, with D(0..3) interleaved (their attnT half-0 data is ready)
    emit_C(1, {0: [lambda: emit_D1(0)],
               2: [lambda: emit_D1(1)],
               3: [lambda: emit_D3a(0, False)],
               4: [lambda: emit_D3b(0, False)],
               5: [lambda: emit_D1(2)],
               6: [lambda: emit_D3a(1, False)],
               7: [lambda: emit_D3b(1, False)],
               8: [lambda: emit_D1(3)],
               9: [lambda: emit_D3a(2, False)],
               10: [lambda: emit_D3b(2, False)],
               12: [lambda: emit_D3a(3, False)],
               13: [lambda: emit_D3b(3, False)]})

    # drain: scatters first (gpsimd runs ahead), D3s pipelined behind
    emit_D1(4)
    emit_D1(5)
    emit_D1(6)
    emit_D1(7)
    emit_D3a(4, True)
    emit_D3b(4, True)
    emit_D3a(5, True)
    emit_D3b(5, True)
    emit_D3a(6, True)
    emit_D3b(6, True)
    emit_D3a(7, True)
    emit_D3b(7, True)


_PROGRAM = None


def get_program():
    global _PROGRAM
    if _PROGRAM is None:
        nc = bacc.Bacc("TRN2", target_bir_lowering=False, debug=False,
                       enable_asserts=False, num_devices=8)
        from contextlib import ExitStack
        with tile.TileContext(nc) as tc, ExitStack() as ctx:
            _emit(nc, tc, ctx)
        nc.compile()
        _PROGRAM = nc
    return _PROGRAM


def make_in_maps(query, key, value, Wg, Wk, bk, Wv, bv, Wq, bq, Wo, bo):
    import ml_dtypes
    BFNP = ml_dtypes.bfloat16

    def hilo(x):
        x = np.asarray(x, np.float32)
        hi = x.astype(BFNP)
        lo = (x - hi.astype(np.float32)).astype(BFNP)
        return hi, lo

    Wg_hi, Wg_lo = hilo(Wg)
    # kh pre-scaled by 1/8 == 1/sqrt(DK); doubled for the two head row-groups
    Wk2s = np.concatenate([np.asarray(Wk), np.asarray(Wk)], axis=1) * 0.125
    # bk shifts all logits of a query equally -> softmax-invariant: dropped.
    # (d e) ordering (col/row index = d*E + e)
    Wq_f = np.asarray(Wq).transpose(1, 2, 0).reshape(D, DK * E)
    Wo_f = np.asarray(Wo).transpose(1, 0, 2).reshape(DK * E, D)
    bq_f = np.asarray(bq).T.reshape(DK * E)
    # bv folds into bo since attention weights sum to 1
    bo_p = np.asarray(bo) + np.einsum('d,edm->em', np.asarray(bv, np.float32),
                                      np.asarray(Wo, np.float32))

    w0a = np.zeros((P, _W0A_COLS), BFNP)
    def put(name, rows, arr):
        base = _W0A[name]
        arr = np.asarray(arr, BFNP)
        w0a[rows, base:base + arr.shape[-1]] = arr
    for dc in range(DC):
        w0a[:, _W0A["Wk2s"] + dc * P:_W0A["Wk2s"] + (dc + 1) * P] = \
            np.asarray(Wk2s[dc * P:(dc + 1) * P, :], BFNP)
        w0a[:, _W0A["Wv"] + dc * DK:_W0A["Wv"] + (dc + 1) * DK] = \
            np.asarray(Wv, np.float32)[dc * P:(dc + 1) * P, :].astype(BFNP)
        w0a[:, _W0A["Wg_hi"] + dc * E:_W0A["Wg_hi"] + (dc + 1) * E] = \
            Wg_hi[dc * P:(dc + 1) * P, :]
        w0a[:, _W0A["Wg_lo"] + dc * E:_W0A["Wg_lo"] + (dc + 1) * E] = \
            Wg_lo[dc * P:(dc + 1) * P, :]
    w0a[:, _W0A["ones"]:_W0A["ones"] + P] = np.ones((P, P), BFNP)
    put("bq_row", 0, bq_f)
    put("bo", slice(0, E), bo_p)

    w0b = np.zeros((P, _W0B_COLS), BFNP)
    for dc in range(DC):
        w0b[:, dc * D:(dc + 1) * D] = \
            np.asarray(Wq_f, np.float32)[dc * P:(dc + 1) * P, :].astype(BFNP)
        w0b[:, DC * D + dc * D:DC * D + (dc + 1) * D] = \
            np.asarray(Wo_f, np.float32)[dc * P:(dc + 1) * P, :].astype(BFNP)

    def pack_chunks(x, n):  # [512, N] -> [128, 4*N] (dc-major columns)
        out = np.empty((P, DC * n), x.dtype)
        for dc in range(DC):
            out[:, dc * n:(dc + 1) * n] = x[dc * P:(dc + 1) * P, :]
        return np.ascontiguousarray(out)

    def pack_blocks(x, nblk, blkcols):  # [512, N] -> [128, nblk*(4*blkcols)]
        # block j = [dc0 cols | dc1 cols | dc2 cols | dc3 cols]
        out = np.empty((P, DC * nblk * blkcols), x.dtype)
        for j in range(nblk):
            for dc in range(DC):
                off = j * DC * blkcols + dc * blkcols
                out[:, off:off + blkcols] = \
                    x[dc * P:(dc + 1) * P, j * blkcols:(j + 1) * blkcols]
        return np.ascontiguousarray(out)

    shared = {"w0a": np.ascontiguousarray(w0a), "w0b": np.ascontiguousarray(w0b)}
    in_maps = []
    for b in range(4):
        kT = np.asarray(key[b], np.float32).T.astype(BFNP)     # [512, 2048]
        vT = np.asarray(value[b], np.float32).T.astype(BFNP)
        kTn = pack_blocks(kT, 4, HD)      # A1-chunk-major blocks of 512 keys
        vTn = pack_blocks(vT, NKC, P)     # kc-major blocks of 128 keys
        for h in range(2):
            qhi, qlo = hilo(np.asarray(query[b][h * NQ:(h + 1) * NQ, :]).T)
            in_maps.append({"kTn": kTn, "vTn": vTn,
                            "qhi": pack_chunks(qhi, NQ),
                            "qlo": pack_chunks(qlo, NQ), **shared})
    return in_maps


def kernel(query, key, value, Wg, Wk, bk, Wv, bv, Wq, bq, Wo, bo):
    in_maps = make_in_maps(query, key, value, Wg, Wk, bk, Wv, bv, Wq, bq, Wo, bo)
    nc = get_program()
    res = bass_utils.run_bass_kernel_spmd(nc, in_maps, core_ids=list(range(8)))
    outs = [np.asarray(res.results[c]["out"], np.float32) for c in range(8)]
    return np.concatenate(outs, axis=0).reshape(4, T, D)
